# revision 1
# baseline (speedup 1.0000x reference)
"""EdgeGraphConv on 8 Trainium2 NeuronCores.

Distribution: dst-range sharding. Core c owns destination nodes
[c*N/8, (c+1)*N/8). The host groups edges by (core, dst-tile-of-128,
src-chunk) -- a pure index-space binning -- so each core's segment-sum
is fully local and the final output is a concatenation (no
collectives).

Device algorithm per core:
  phase 0: h = node_feat @ W_node for ALL nodes (replicated work),
           stored to a private HBM table (rows padded to 256B, row
           order swizzled so the store DMA is one contiguous run per
           partition). b_node is folded out algebraically (below).
  phase 2: per super-round (R dst tiles) and src-chunk k: one
           dma_gather (int16 chunk-relative indices) pulls h[src] for
           all that round's chunk-k edges into SBUF; per dst tile a
           one-hot (edge -> dst-local-id, iota+is_equal) matmul
           accumulates in PSUM, in one f32 accumulator:
           S = segsum(h[src]), ef_sum = segsum(edge_feat), deg = count.
  final:   out = (S + ef_sum*W_edge + deg*(b_node+b_edge)) / max(deg,1)
           == mean(h[src]+he) with biases restored; exactly 0 for
           isolated nodes.

The schedule (TILES x NCHUNK x B4 blocks) is data-independent given B4,
so one NEFF serves all 8 cores; per-core differences are pure data.
"""

import sys

for _p in ("/opt/trn_rl_repo", "/opt/pypackages"):
    if _p not in sys.path:
        sys.path.append(_p)

from contextlib import ExitStack

import ml_dtypes
import numpy as np

import concourse.bass as bass
import concourse.mybir as mybir
import concourse.tile as tile
from concourse import bacc, library_config
from concourse.bass_utils import run_bass_kernel_spmd

BF16 = ml_dtypes.bfloat16
N_CORES = 8
P = 128
FE = 128           # padded h-table row elements (256 B)
NCHUNK = 4         # src chunks (chunk row count must fit int16)


def build_bass(B4, K_in, F, TILES, TBL_T, R, PH0_TILES, debug_mode=None):
    """Build the single-NEFF 8-core SPMD bass program.

    B4: 128-edge blocks per (dst-tile, src-chunk);  R: dst tiles per
    super-round (TILES % R == 0);  TBL_T: h-table tiles (global nodes
    padded to TBL_T*128;  must be divisible by NCHUNK).
    """
    NBLK = TILES * NCHUNK * B4
    PAD_N = TBL_T * P
    CH = PAD_N // NCHUNK
    assert TILES % R == 0 and PAD_N % NCHUNK == 0 and CH <= 32768
    NR = TILES // R
    CALL_IDX = R * B4 * P           # indices per dma_gather call

    nc = bacc.Bacc("TRN2", target_bir_lowering=False, debug=False,
                   num_devices=N_CORES)
    dt = mybir.dt

    nfT_d = nc.dram_tensor("nft", [K_in, PAD_N], dt.bfloat16, kind="ExternalInput")
    Wn_d = nc.dram_tensor("wn", [K_in, F], dt.bfloat16, kind="ExternalInput")
    we_d = nc.dram_tensor("we", [1, F], dt.float32, kind="ExternalInput")
    bn_d = nc.dram_tensor("bn", [1, F], dt.float32, kind="ExternalInput")
    be_d = nc.dram_tensor("be", [1, F], dt.float32, kind="ExternalInput")
    iot_d = nc.dram_tensor("iot", [1, P], dt.bfloat16, kind="ExternalInput")
    idx_d = nc.dram_tensor("idx", [P, NBLK * 8], dt.int16, kind="ExternalInput")
    dstl_d = nc.dram_tensor("dstl", [P, TILES, NCHUNK, B4], dt.bfloat16,
                            kind="ExternalInput")
    efo_d = nc.dram_tensor("efo", [P, TILES, NCHUNK, B4, 2], dt.bfloat16,
                           kind="ExternalInput")
    out_d = nc.dram_tensor("out", [TILES * P, F], dt.float32, kind="ExternalOutput")

    # h table row rho = (node % 128) * TBL_T + node // 128  (store is one
    # contiguous DRAM run per partition; gather offsets precomputed in
    # rho space, chunk-relative).
    hkind = {"ph0": "ExternalOutput", "ph2": "ExternalInput"}.get(
        debug_mode, "Internal")
    htbl = nc.dram_tensor("htbl", [PAD_N, FE], dt.bfloat16, kind=hkind)
    htbl_v = htbl.ap().rearrange("(p t) f -> p t f", t=TBL_T)

    mult = mybir.AluOpType.mult
    is_equal = mybir.AluOpType.is_equal

    def emit_phase0(tc):
        with tc.tile_pool(name="ph0", bufs=2) as p0, \
             tc.tile_pool(name="ph0w", bufs=1) as p0w, \
             tc.tile_pool(name="ph0ps", bufs=8, space="PSUM") as p0ps:
            wt = p0w.tile([K_in, F], dt.bfloat16)
            nc.sync.dma_start(out=wt[:], in_=Wn_d.ap())
            for t0 in range(0, TBL_T, PH0_TILES):
                nt = min(PH0_TILES, TBL_T - t0)
                nf_t = p0.tile([K_in, PH0_TILES * P], dt.bfloat16, tag="nf")
                nc.sync.dma_start(out=nf_t[:, :nt * P],
                                  in_=nfT_d.ap()[:, t0 * P:(t0 + nt) * P])
                hst = p0.tile([P, PH0_TILES, FE], dt.bfloat16, tag="hst")
                nc.vector.memset(hst[:, :, F:], 0.0)
                for j0 in range(0, nt, 4):
                    nb = min(4, nt - j0)
                    ps = p0ps.tile([P, 4, F], dt.float32, tag="ps")
                    for j in range(nb):
                        nc.tensor.matmul(
                            ps[:, j, :],
                            lhsT=nf_t[:, (j0 + j) * P:(j0 + j + 1) * P],
                            rhs=wt[:],
                            start=True, stop=True)
                    nc.scalar.copy(out=hst[:, j0:j0 + nb, 0:F],
                                   in_=ps[:, :nb, :])
                nc.sync.dma_start(out=htbl_v[:, t0:t0 + nt, :],
                                  in_=hst[:, :nt, :])

    def emit_phase2(tc, ctx):
        meta = ctx.enter_context(tc.tile_pool(name="meta", bufs=1))
        idx_sb = meta.tile([P, NBLK * 8], dt.int16)
        nc.sync.dma_start(out=idx_sb[:], in_=idx_d.ap())
        dstl_sb = meta.tile([P, TILES, NCHUNK, B4, 1], dt.bfloat16)
        nc.sync.dma_start(out=dstl_sb[:, :, :, :, 0], in_=dstl_d.ap())
        efo_sb = meta.tile([P, TILES, NCHUNK, B4, 2], dt.bfloat16)
        nc.sync.dma_start(out=efo_sb[:], in_=efo_d.ap())

        iota_t = meta.tile([P, 1, 1, P], dt.bfloat16)
        nc.sync.dma_start(out=iota_t[:, 0, :, :],
                          in_=iot_d.ap()[0:1, :].partition_broadcast(P))
        web = meta.tile([P, 1, F], dt.float32)
        nc.sync.dma_start(out=web[:],
                          in_=we_d.ap()[0:1, :].partition_broadcast(P))
        bnb = meta.tile([P, 1, F], dt.float32)
        nc.sync.dma_start(out=bnb[:],
                          in_=bn_d.ap()[0:1, :].partition_broadcast(P))
        beb = meta.tile([P, 1, F], dt.float32)
        nc.sync.dma_start(out=beb[:],
                          in_=be_d.ap()[0:1, :].partition_broadcast(P))
        bb = meta.tile([P, 1, F], dt.float32)
        nc.vector.tensor_add(out=bb[:], in0=bnb[:], in1=beb[:])

        acc = meta.tile([P, TILES, F + 2], dt.float32)

        nc.gpsimd.load_library(library_config.mlp)

        with tc.tile_pool(name="p2", bufs=2) as p2, \
             tc.tile_pool(name="p2oh", bufs=4) as p2oh, \
             tc.tile_pool(name="p2ps", bufs=4, space="PSUM") as p2ps:
            for r in range(NR):
                t0 = r * R
                stages = []
                for k in range(NCHUNK):
                    st = p2.tile([P, R * B4, FE], dt.bfloat16, tag=f"st{k}")
                    col0 = (r * NCHUNK + k) * (CALL_IDX // 16)
                    # SWDGE ring holds ~1024 descriptors per shot; split.
                    SUB = 1024
                    for s0 in range(0, CALL_IDX, SUB):
                        ns = min(SUB, CALL_IDX - s0)
                        nc.gpsimd.dma_gather(
                            out_ap=st[:, s0 // P:(s0 + ns) // P, :],
                            in_ap=htbl.ap()[k * CH:(k + 1) * CH, :],
                            idxs_ap=idx_sb[:, col0 + s0 // 16:
                                           col0 + (s0 + ns) // 16],
                            num_idxs=ns, num_idxs_reg=ns,
                            elem_size=FE)
                    st_v = st[:].rearrange("p (t b) f -> p t b f", b=B4)
                    nc.vector.tensor_copy(
                        out=st_v[:, :, :, F:F + 2],
                        in_=efo_sb[:, t0:t0 + R, k, :, :])
                    stages.append(st)
                for tt in range(R):
                    t = t0 + tt
                    oh = p2oh.tile([P, NCHUNK, B4, P], dt.bfloat16, tag="oh")
                    nc.vector.tensor_tensor(
                        out=oh[:],
                        in0=dstl_sb[:, t, :, :, :].to_broadcast(
                            [P, NCHUNK, B4, P]),
                        in1=iota_t[:].to_broadcast(
                            [P, NCHUNK, B4, P]),
                        op=is_equal)
                    ps2 = p2ps.tile([P, F + 2], dt.float32, tag="ps2")
                    for k in range(NCHUNK):
                        st = stages[k]
                        for b in range(B4):
                            c = tt * B4 + b
                            nc.tensor.matmul(
                                ps2[:],
                                lhsT=oh[:, k, b, :],
                                rhs=st[:, c, 0:F + 2],
                                start=(k == 0 and b == 0),
                                stop=(k == NCHUNK - 1 and b == B4 - 1))
                    nc.scalar.copy(out=acc[:, t, :], in_=ps2[:])

        with tc.tile_pool(name="fin", bufs=1) as fin:
            S = acc[:, :, 0:F]
            ef = acc[:, :, F:F + 1]
            dg = acc[:, :, F + 1:F + 2]
            md = fin.tile([P, TILES, 1], dt.float32)
            nc.vector.tensor_scalar_max(md[:], dg, 1.0)
            rcp = fin.tile([P, TILES, 1], dt.float32)
            nc.vector.reciprocal(out=rcp[:], in_=md[:])
            t1 = fin.tile([P, TILES, F], dt.float32)
            nc.vector.tensor_tensor(out=t1[:],
                                    in0=ef.to_broadcast([P, TILES, F]),
                                    in1=web[:].to_broadcast([P, TILES, F]),
                                    op=mult)
            nc.vector.tensor_add(out=t1[:], in0=t1[:], in1=S)
            t2 = fin.tile([P, TILES, F], dt.float32)
            nc.vector.tensor_tensor(out=t2[:],
                                    in0=dg.to_broadcast([P, TILES, F]),
                                    in1=bb[:].to_broadcast([P, TILES, F]),
                                    op=mult)
            nc.vector.tensor_add(out=t1[:], in0=t1[:], in1=t2[:])
            nc.vector.tensor_tensor(out=t1[:], in0=t1[:],
                                    in1=rcp[:].to_broadcast([P, TILES, F]),
                                    op=mult)
            nc.sync.dma_start(
                out=out_d.ap().rearrange("(p t) f -> p t f", t=TILES),
                in_=t1[:])

    with tile.TileContext(nc) as tc, ExitStack() as ctx:
        if debug_mode != "ph2":
            emit_phase0(tc)
        if debug_mode != "ph0":
            emit_phase2(tc, ctx)
    nc.compile()
    return nc


def _schedule(src, dst, edge_feat, n_nodes, B_override=None):
    """Host-side index-space binning by (core, dst-tile, src-chunk)."""
    E = src.shape[0]
    RN = n_nodes // N_CORES
    TILES = (RN + P - 1) // P
    TBL_T = -(-(n_nodes) // P)
    TBL_T = -(-TBL_T // NCHUNK) * NCHUNK        # divisible by NCHUNK
    PAD_N = TBL_T * P
    CH = PAD_N // NCHUNK

    rho = (src % P) * TBL_T + src // P          # table row of each src
    k = rho // CH
    core = dst // RN
    L = dst - core * RN
    t = L // P
    u = (L % P).astype(np.float32)
    bins = (core * TILES + t) * NCHUNK + k
    nbins = N_CORES * TILES * NCHUNK
    cnt = np.bincount(bins, minlength=nbins)
    B4 = max(1, int(np.max((cnt + P - 1) // P)))
    if B_override is not None:
        B4 = max(B4, B_override)

    order = np.argsort(bins, kind="stable")
    bin_start = np.zeros(nbins, dtype=np.int64)
    np.cumsum(cnt[:-1], out=bin_start[1:])
    rank = np.arange(E, dtype=np.int64) - bin_start[bins[order]]
    dest = bins[order] * (B4 * P) + rank

    SLOTS = nbins * B4 * P
    idxv = np.zeros(SLOTS, dtype=np.int16)         # pad: chunk row 0
    dstl = np.full(SLOTS, -1.0, dtype=np.float32)  # pad: no iota match
    efv = np.zeros(SLOTS, dtype=np.float32)
    one = np.zeros(SLOTS, dtype=np.float32)

    idxv[dest] = (rho - k * CH)[order].astype(np.int16)
    dstl[dest] = u[order]
    efv[dest] = edge_feat[order, 0]
    one[dest] = 1.0

    NBLK = TILES * NCHUNK * B4
    per_core = []
    for c in range(N_CORES):
        sl = slice(c * NBLK * P, (c + 1) * NBLK * P)
        iv = idxv[sl].reshape(TILES, NCHUNK, B4 * P)
        dl = dstl[sl].reshape(TILES, NCHUNK, B4, P).transpose(3, 0, 1, 2)
        eo = np.stack([efv[sl], one[sl]], axis=-1)
        eo = eo.reshape(TILES, NCHUNK, B4, P, 2).transpose(3, 0, 1, 2, 4)
        per_core.append((iv, dl.astype(BF16).copy(), eo.astype(BF16).copy()))
    return per_core, B4, TILES, TBL_T, RN


def _pack_idx(iv, TILES, B4, R):
    """[TILES, NCHUNK, B4*P] chunk-relative rows -> wrapped [P, NBLK*8]."""
    NR = TILES // R
    segs = []
    for r in range(NR):
        for k in range(NCHUNK):
            seq = iv[r * R:(r + 1) * R, k, :].reshape(-1)     # R*B4*128
            segs.append(np.tile(seq.reshape(-1, 16).T, (8, 1)))
    return np.concatenate(segs, axis=1).astype(np.int16)


def _run(node_feat, edge_feat, W_node, b_node, W_edge, b_edge, src, dst,
         r_pref=7, ph0_tiles=98, trace=False, debug_mode=None,
         htbl_in=None):
    n_nodes, K_in = node_feat.shape
    F = W_node.shape[1]
    src = np.asarray(src, dtype=np.int64)
    dst = np.asarray(dst, dtype=np.int64)

    per_core, B4, TILES, TBL_T, RN = _schedule(src, dst, edge_feat, n_nodes)
    R = 1
    for d in range(1, TILES + 1):
        if TILES % d == 0 and d <= r_pref:
            R = d
    PAD_N = TBL_T * P

    nc = build_bass(B4, K_in, F, TILES, TBL_T, R, min(ph0_tiles, TBL_T),
                    debug_mode=debug_mode)

    nfT = np.zeros((K_in, PAD_N), dtype=BF16)
    nfT[:, :n_nodes] = node_feat.T.astype(BF16)
    base = {
        "nft": nfT,
        "wn": W_node.astype(BF16),
        "we": W_edge.astype(np.float32).reshape(1, F),
        "bn": b_node.astype(np.float32).reshape(1, F),
        "be": b_edge.astype(np.float32).reshape(1, F),
        "iot": np.arange(P, dtype=np.float32).reshape(1, P).astype(BF16),
    }
    in_maps = []
    for c in range(N_CORES):
        iv, dl, eo = per_core[c]
        m = dict(base)
        m["idx"] = _pack_idx(iv, TILES, B4, R)
        m["dstl"] = dl
        m["efo"] = eo
        if debug_mode == "ph2":
            m["htbl"] = htbl_in
        in_maps.append(m)

    res = run_bass_kernel_spmd(nc, in_maps, core_ids=list(range(N_CORES)),
                               trace=trace)
    if debug_mode == "ph0":
        return None, res

    # unswizzle: core output row p*TILES + t  ->  local node t*128 + p
    loc = np.arange(RN, dtype=np.int64)
    rows = (loc % P) * TILES + loc // P
    out = np.empty((n_nodes, F), dtype=np.float32)
    for c in range(N_CORES):
        out[c * RN:(c + 1) * RN] = res.results[c]["out"][rows]
    return out, res


def kernel(node_feat, edge_feat, W_node, b_node, W_edge, b_edge, src, dst):
    out, _ = _run(node_feat, edge_feat, W_node, b_node, W_edge, b_edge,
                  src, dst)
    return out



# revision 4
# speedup vs baseline: 3.2855x; 3.2855x over previous
"""EdgeGraphConv on 8 Trainium2 NeuronCores (v2: raw-feature gather).

Distribution: dst-range sharding. Core c owns destination nodes
[c*N/8, (c+1)*N/8). No collectives; output is a concatenation.

Linearity trick: mean_e(h0[src]+bn + ef*We+be) over dst =
  (S_raw @ W + EF*We + deg*(bn+be)) / max(deg,1)
where S_raw[dst] = sum nf[src_e]  (raw 128-dim features!),
      EF[dst]    = sum ef_e,  deg = in-degree (host metadata).
So there is NO per-node pre-matmul (old phase 0 deleted): the kernel
gathers raw node-feature rows (256 B each, fully used) and applies
W_node per dst tile AFTER aggregation (2 tiny matmuls per tile).

Gather: SWDGE dma_gather of 256B rows, round-robin over 4 SWDGE
queues (4 descriptor rings on distinct Q7 core pairs) so descriptor
generation and ring drain overlap; measured ~2.6 ns/descriptor.

Binning: per (core, dst-tile), edges split into 4 src-value quarters
(window < 32768 rows for chunk-relative int16 indices), each bin
padded to B4 128-slot blocks. Segment-sum per tile via one-hot
(iota+is_equal) matmuls accumulating raw sums [128] in PSUM. ef-sums
via a host-placed [dst, LMAX] grid + one DVE reduce. deg/rdeg are
host-computed index metadata.

Single NEFF serves all 8 cores (per-core differences are pure data).
"""

import sys

for _p in ("/opt/trn_rl_repo", "/opt/pypackages"):
    if _p not in sys.path:
        sys.path.append(_p)

import ml_dtypes
import numpy as np

import concourse.bass as bass
import concourse.mybir as mybir
import concourse.tile as tile
from concourse import bacc, library_config
from concourse.bass_utils import run_bass_kernel_spmd

BF16 = ml_dtypes.bfloat16
N_CORES = 8
P = 128
NGRP = 4           # src-value quarters (windows of CH rows)
SUB = 2048         # idxs per dma_gather call (128 descs/engine)
NQ = 4             # SWDGE queues


def build_bass(K_in, F, TILES, R, B4, LMAX, PAD_N, CH, debug=False):
    NR = TILES // R
    TPB = NGRP * B4                          # blocks per tile
    SLOTS_T = TPB * P                        # slots per tile
    TOT_SLOTS = TILES * SLOTS_T
    RBLK = R * TPB                           # st blocks per round

    nc = bacc.Bacc("TRN2", target_bir_lowering=False, debug=False,
                   num_devices=N_CORES, num_swdge_queues=NQ)
    dt = mybir.dt

    nfr_d = nc.dram_tensor("nfr", [PAD_N, K_in], dt.bfloat16, kind="ExternalInput")
    wn_d = nc.dram_tensor("wn", [K_in, F], dt.bfloat16, kind="ExternalInput")
    idn_d = nc.dram_tensor("idn", [P, P], dt.bfloat16, kind="ExternalInput")
    iot_d = nc.dram_tensor("iot", [1, P], dt.bfloat16, kind="ExternalInput")
    we_d = nc.dram_tensor("we", [1, F], dt.float32, kind="ExternalInput")
    bb_d = nc.dram_tensor("bb", [1, F], dt.float32, kind="ExternalInput")
    idx_d = nc.dram_tensor("idx", [P, TOT_SLOTS // 16], dt.int16,
                           kind="ExternalInput")
    dstl_d = nc.dram_tensor("dstl", [P, TILES, TPB], dt.bfloat16,
                            kind="ExternalInput")
    efg_d = nc.dram_tensor("efg", [P, TILES, LMAX], dt.bfloat16,
                           kind="ExternalInput")
    dg_d = nc.dram_tensor("dg", [P, TILES, 2], dt.float32, kind="ExternalInput")
    out_d = nc.dram_tensor("out", [TILES * P, F], dt.float32,
                           kind="ExternalOutput")

    mult = mybir.AluOpType.mult
    is_equal = mybir.AluOpType.is_equal

    def st_blk(g, tt, b):
        return g * (R * B4) + tt * B4 + b

    with tile.TileContext(nc) as tc:
        with tc.tile_pool(name="meta", bufs=1) as meta, \
             tc.tile_pool(name="st", bufs=2) as pst, \
             tc.tile_pool(name="oh", bufs=4) as poh, \
             tc.tile_pool(name="fin", bufs=2) as pfin, \
             tc.tile_pool(name="ps", bufs=3, space="PSUM") as pps, \
             tc.tile_pool(name="psT", bufs=2, space="PSUM") as ppsT, \
             tc.tile_pool(name="pso", bufs=2, space="PSUM") as ppso:
            idx_sb = meta.tile([P, TOT_SLOTS // 16], dt.int16)
            nc.sync.dma_start(out=idx_sb[:], in_=idx_d.ap())
            dstl_sb = meta.tile([P, TILES, TPB, 1], dt.bfloat16)
            nc.sync.dma_start(out=dstl_sb[:, :, :, 0], in_=dstl_d.ap())
            efg_sb = meta.tile([P, TILES, LMAX], dt.bfloat16)
            nc.sync.dma_start(out=efg_sb[:], in_=efg_d.ap())
            dg_sb = meta.tile([P, TILES, 2], dt.float32)
            nc.sync.dma_start(out=dg_sb[:], in_=dg_d.ap())
            wn_sb = meta.tile([K_in, F], dt.bfloat16)
            nc.sync.dma_start(out=wn_sb[:], in_=wn_d.ap())
            idn_sb = meta.tile([P, P], dt.bfloat16)
            nc.sync.dma_start(out=idn_sb[:], in_=idn_d.ap())
            iota_t = meta.tile([P, 1, P], dt.bfloat16)
            nc.sync.dma_start(out=iota_t[:, 0, :],
                              in_=iot_d.ap()[0:1, :].partition_broadcast(P))
            web = meta.tile([P, 1, F], dt.float32)
            nc.sync.dma_start(out=web[:],
                              in_=we_d.ap()[0:1, :].partition_broadcast(P))
            bbb = meta.tile([P, 1, F], dt.float32)
            nc.sync.dma_start(out=bbb[:],
                              in_=bb_d.ap()[0:1, :].partition_broadcast(P))

            acc = meta.tile([P, TILES, K_in], dt.bfloat16)
            efs = meta.tile([P, TILES, 1], dt.float32)
            osb = meta.tile([P, TILES, F], dt.float32)

            nc.gpsimd.load_library(library_config.mlp)

            # EF[dst] = sum of this dst's edge_feat values (grid is
            # host-placed, zero-padded to LMAX per node).
            nc.vector.tensor_reduce(out=efs[:], in_=efg_sb[:],
                                    axis=mybir.AxisListType.X,
                                    op=mybir.AluOpType.add)

            qc = 0
            col = 0
            for r in range(NR):
                st = pst.tile([P, RBLK, K_in], dt.bfloat16, tag="st")
                for g in range(NGRP):
                    n_g = R * B4 * P
                    g0 = st_blk(g, 0, 0)
                    base = g * CH
                    for s0 in range(0, n_g, SUB):
                        ns = min(SUB, n_g - s0)
                        nc.gpsimd.dma_gather(
                            out_ap=st[:, g0 + s0 // P: g0 + (s0 + ns) // P, :],
                            in_ap=nfr_d.ap()[base:PAD_N, :],
                            idxs_ap=idx_sb[:, col + s0 // 16:
                                           col + (s0 + ns) // 16],
                            num_idxs=ns, num_idxs_reg=ns,
                            elem_size=K_in,
                            queue_num=qc % NQ,
                            single_packet=(ns <= 1024))
                        qc += 1
                    col += n_g // 16
                for tt in range(R):
                    t = r * R + tt
                    oh = poh.tile([P, TPB, P], dt.bfloat16, tag="oh")
                    nc.vector.tensor_tensor(
                        out=oh[:],
                        in0=dstl_sb[:, t, :, :].to_broadcast([P, TPB, P]),
                        in1=iota_t[:].to_broadcast([P, TPB, P]),
                        op=is_equal)
                    ps = pps.tile([P, K_in], dt.float32, tag="ps")
                    for j in range(TPB):
                        g, b = divmod(j, B4)
                        nc.tensor.matmul(
                            ps[:],
                            lhsT=oh[:, j, :],
                            rhs=st[:, st_blk(g, tt, b), :],
                            start=(j == 0), stop=(j == TPB - 1))
                    nc.scalar.copy(out=acc[:, t, :], in_=ps[:])
                    # finalize tile t inline: S_raw^T then @ W_node
                    psT = ppsT.tile([P, P], dt.float32, tag="psT")
                    nc.tensor.matmul(psT[:], lhsT=acc[:, t, :], rhs=idn_sb[:],
                                     start=True, stop=True)
                    hT = pfin.tile([P, P], dt.bfloat16, tag="hT")
                    nc.scalar.copy(out=hT[:], in_=psT[:])
                    pso = ppso.tile([P, F], dt.float32, tag="pso")
                    nc.tensor.matmul(pso[:], lhsT=hT[:], rhs=wn_sb[:],
                                     start=True, stop=True)
                    nc.scalar.copy(out=osb[:, t, :], in_=pso[:])

            # epilogue: out = (S@W + EF*We + deg*(bn+be)) * rdeg
            t1 = meta.tile([P, TILES, F], dt.float32)
            nc.vector.tensor_tensor(out=t1[:],
                                    in0=efs[:].to_broadcast([P, TILES, F]),
                                    in1=web[:].to_broadcast([P, TILES, F]),
                                    op=mult)
            nc.vector.tensor_add(out=osb[:], in0=osb[:], in1=t1[:])
            nc.vector.tensor_tensor(
                out=t1[:],
                in0=dg_sb[:, :, 0:1].to_broadcast([P, TILES, F]),
                in1=bbb[:].to_broadcast([P, TILES, F]),
                op=mult)
            nc.vector.tensor_add(out=osb[:], in0=osb[:], in1=t1[:])
            nc.vector.tensor_tensor(
                out=osb[:], in0=osb[:],
                in1=dg_sb[:, :, 1:2].to_broadcast([P, TILES, F]),
                op=mult)
            nc.sync.dma_start(
                out=out_d.ap().rearrange("(p t) f -> p t f", t=TILES),
                in_=osb[:])
    nc.compile()
    return nc


def _schedule(src, dst, edge_feat, n_nodes):
    """Host-side index-space binning by (core, dst-tile, src-quarter)."""
    RN = n_nodes // N_CORES
    TILES = (RN + P - 1) // P
    R = 1
    for d in range(1, TILES + 1):
        if TILES % d == 0 and d <= 7:
            R = d
    NR = TILES // R
    PAD_N = -(-n_nodes // P) * P
    CH = -(-PAD_N // NGRP)
    assert CH <= 32768

    core = dst // RN
    L = dst - core * RN
    t = L // P
    u = (L % P).astype(np.float32)
    g = src // CH
    key = (core * TILES + t) * NGRP + g
    order = np.lexsort((src, key))
    ks, ss, us = key[order], src[order], u[order]
    nbins = N_CORES * TILES * NGRP
    cnt = np.bincount(key, minlength=nbins)
    starts = np.zeros(nbins + 1, dtype=np.int64)
    np.cumsum(cnt, out=starts[1:])
    B4 = max(1, int(np.ceil(cnt.max() / P)))
    TPB = NGRP * B4
    SLOTS_T = TPB * P

    per_core = []
    for c in range(N_CORES):
        idxv = np.zeros(TILES * SLOTS_T, dtype=np.int16)
        dstl = np.full((TILES, TPB, P), -1.0, dtype=np.float32)
        pos = 0
        # gather-stream order: [round][quarter][tile][B4*P slots]
        for r in range(NR):
            for gg in range(NGRP):
                for tt in range(R):
                    bi = (c * TILES + r * R + tt) * NGRP + gg
                    a, b = starts[bi], starts[bi + 1]
                    n = b - a
                    idxv[pos:pos + n] = (ss[a:b] - gg * CH).astype(np.int16)
                    blkv = np.full(B4 * P, -1.0, dtype=np.float32)
                    blkv[:n] = us[a:b]
                    dstl[r * R + tt, gg * B4:(gg + 1) * B4, :] = \
                        blkv.reshape(B4, P)
                    pos += B4 * P
        per_core.append((idxv, dstl.transpose(2, 0, 1).astype(BF16).copy()))
    return per_core, TILES, R, B4, TPB, RN, PAD_N, CH


def _pack_idx(idxv):
    """flat slot-ordered int16 idxs -> wrapped [P, n/16] (16-partition
    wrap, replicated to the 8 16-partition groups)."""
    w = idxv.reshape(-1, 16).T           # [16, n/16]
    return np.tile(w, (8, 1)).astype(np.int16)


def _run(node_feat, edge_feat, W_node, b_node, W_edge, b_edge, src, dst,
         trace=False):
    n_nodes, K_in = node_feat.shape
    F = W_node.shape[1]
    src = np.asarray(src, dtype=np.int64)
    dst = np.asarray(dst, dtype=np.int64)
    E = src.shape[0]

    per_core, TILES, R, B4, TPB, NR, PAD_N, CH = \
        _schedule(src, dst, edge_feat, n_nodes)
    RN = n_nodes // N_CORES

    nfr = np.zeros((PAD_N, K_in), dtype=BF16)
    nfr[:n_nodes] = node_feat.astype(BF16)

    # per-dst ef grid + degree (host index metadata). Device row of
    # global node n is core*(TILES*P) + (n - core*RN) -- note the
    # per-core tile padding (TILES*P >= RN).
    deg = np.bincount(dst, minlength=n_nodes).astype(np.int64)
    LMAX = max(1, int(deg.max()))
    do = np.argsort(dst, kind="stable")
    dstart = np.zeros(n_nodes + 1, dtype=np.int64)
    np.cumsum(deg, out=dstart[1:])
    rank = np.arange(E, dtype=np.int64) - dstart[dst[do]]
    nn = np.arange(n_nodes, dtype=np.int64)
    pos = (nn // RN) * (TILES * P) + nn % RN      # node -> device row
    grid = np.zeros((N_CORES * TILES * P, LMAX), dtype=np.float32)
    grid[pos[dst[do]], rank] = edge_feat[do, 0]
    degp = np.zeros(N_CORES * TILES * P, dtype=np.float32)
    degp[pos] = deg

    nc = build_bass(K_in, F, TILES, R, B4, LMAX, PAD_N, CH)

    base_in = {
        "nfr": nfr,
        "wn": W_node.astype(BF16),
        "idn": np.eye(P, dtype=np.float32).astype(BF16),
        "iot": np.arange(P, dtype=np.float32).reshape(1, P).astype(BF16),
        "we": W_edge.astype(np.float32).reshape(1, F),
        "bb": (b_node + b_edge).astype(np.float32).reshape(1, F),
    }
    in_maps = []
    for c in range(N_CORES):
        idxv, dstp = per_core[c]
        m = dict(base_in)
        m["idx"] = _pack_idx(idxv)
        m["dstl"] = dstp
        gr = grid[c * TILES * P:(c + 1) * TILES * P]
        m["efg"] = gr.reshape(TILES, P, LMAX).transpose(1, 0, 2) \
                     .astype(BF16).copy()
        d = degp[c * TILES * P:(c + 1) * TILES * P].reshape(TILES, P)
        dgm = np.stack([d.T, 1.0 / np.maximum(d.T, 1.0)], axis=-1)
        m["dg"] = np.ascontiguousarray(dgm.astype(np.float32))
        in_maps.append(m)

    res = run_bass_kernel_spmd(nc, in_maps, core_ids=list(range(N_CORES)),
                               trace=trace)
    loc = np.arange(RN, dtype=np.int64)
    rows = (loc % P) * TILES + loc // P
    out = np.empty((n_nodes, F), dtype=np.float32)
    for c in range(N_CORES):
        out[c * RN:(c + 1) * RN] = res.results[c]["out"][rows]
    return out, res


def kernel(node_feat, edge_feat, W_node, b_node, W_edge, b_edge, src, dst):
    out, _ = _run(node_feat, edge_feat, W_node, b_node, W_edge, b_edge,
                  src, dst)
    return out


# revision 7
# speedup vs baseline: 3.3289x; 1.0132x over previous
"""EdgeGraphConv on 8 Trainium2 NeuronCores (v2: raw-feature gather).

Distribution: dst-range sharding. Core c owns destination nodes
[c*N/8, (c+1)*N/8). No collectives; output is a concatenation.

Linearity trick: mean_e(h0[src]+bn + ef*We+be) over dst =
  (S_raw @ W + EF*We + deg*(bn+be)) / max(deg,1)
where S_raw[dst] = sum nf[src_e]  (raw 128-dim features!),
      EF[dst]    = sum ef_e,  deg = in-degree (host metadata).
So there is NO per-node pre-matmul (old phase 0 deleted): the kernel
gathers raw node-feature rows (256 B each, fully used) and applies
W_node per dst tile AFTER aggregation (2 tiny matmuls per tile).

Gather: SWDGE dma_gather of 256B rows, round-robin over 4 SWDGE
queues (4 descriptor rings on distinct Q7 core pairs) so descriptor
generation and ring drain overlap; measured ~2.6 ns/descriptor.

Binning: per (core, dst-tile), edges split into 4 src-value quarters
(window < 32768 rows for chunk-relative int16 indices), each bin
padded to B4 128-slot blocks. Segment-sum per tile via one-hot
(iota+is_equal) matmuls accumulating raw sums [128] in PSUM. ef-sums
via a host-placed [dst, LMAX] grid + one DVE reduce. deg/rdeg are
host-computed index metadata.

Single NEFF serves all 8 cores (per-core differences are pure data).
"""

import sys

for _p in ("/opt/trn_rl_repo", "/opt/pypackages"):
    if _p not in sys.path:
        sys.path.append(_p)

import ml_dtypes
import numpy as np

import concourse.bass as bass
import concourse.mybir as mybir
import concourse.tile as tile
from concourse import bacc, library_config
from concourse.bass_utils import run_bass_kernel_spmd

BF16 = ml_dtypes.bfloat16
N_CORES = 8
P = 128
NGRP = 4           # src-value quarters (windows of CH rows)
SUB = 2048         # idxs per dma_gather call (128 descs/engine)
NQ = 4             # SWDGE queues


def build_bass(K_in, F, TILES, R, B4, LMAX, PAD_N, CH, debug=False):
    NR = TILES // R
    TPB = NGRP * B4                          # blocks per tile
    SLOTS_T = TPB * P                        # slots per tile
    TOT_SLOTS = TILES * SLOTS_T
    RBLK = R * TPB                           # st blocks per round

    nc = bacc.Bacc("TRN2", target_bir_lowering=False, debug=False,
                   num_devices=N_CORES, num_swdge_queues=NQ)
    dt = mybir.dt

    nfr_d = nc.dram_tensor("nfr", [PAD_N, K_in], dt.bfloat16, kind="ExternalInput")
    wn_d = nc.dram_tensor("wn", [K_in, F], dt.bfloat16, kind="ExternalInput")
    idn_d = nc.dram_tensor("idn", [P, P], dt.bfloat16, kind="ExternalInput")
    iot_d = nc.dram_tensor("iot", [1, P], dt.bfloat16, kind="ExternalInput")
    we_d = nc.dram_tensor("we", [1, F], dt.float32, kind="ExternalInput")
    bb_d = nc.dram_tensor("bb", [1, F], dt.float32, kind="ExternalInput")
    idx_d = nc.dram_tensor("idx", [P, TOT_SLOTS // 16], dt.int16,
                           kind="ExternalInput")
    dstl_d = nc.dram_tensor("dstl", [P, TILES, TPB], dt.bfloat16,
                            kind="ExternalInput")
    efg_d = nc.dram_tensor("efg", [P, TILES, LMAX], dt.bfloat16,
                           kind="ExternalInput")
    dg_d = nc.dram_tensor("dg", [P, TILES, 2], dt.float32, kind="ExternalInput")
    out_d = nc.dram_tensor("out", [TILES * P, F], dt.float32,
                           kind="ExternalOutput")

    mult = mybir.AluOpType.mult
    is_equal = mybir.AluOpType.is_equal

    def st_blk(g, tt, b):
        return g * (R * B4) + tt * B4 + b

    with tile.TileContext(nc) as tc:
        with tc.tile_pool(name="meta", bufs=1) as meta, \
             tc.tile_pool(name="st", bufs=2) as pst, \
             tc.tile_pool(name="oh", bufs=4) as poh, \
             tc.tile_pool(name="fin", bufs=2) as pfin, \
             tc.tile_pool(name="ps", bufs=3, space="PSUM") as pps, \
             tc.tile_pool(name="psT", bufs=2, space="PSUM") as ppsT, \
             tc.tile_pool(name="pso", bufs=2, space="PSUM") as ppso:
            nc.gpsimd.load_library(library_config.mlp)
            RCOL = SLOTS_T * R // 16          # idx columns per round
            idx_sb = meta.tile([P, TOT_SLOTS // 16], dt.int16)
            nc.sync.dma_start(out=idx_sb[:, 0:RCOL], in_=idx_d.ap()[:, 0:RCOL])
            dstl_sb = meta.tile([P, TILES, TPB, 1], dt.bfloat16)
            nc.sync.dma_start(out=dstl_sb[:, :, :, 0], in_=dstl_d.ap())
            efg_sb = meta.tile([P, TILES, LMAX], dt.bfloat16)
            nc.sync.dma_start(out=efg_sb[:], in_=efg_d.ap())
            dg_sb = meta.tile([P, TILES, 2], dt.float32)
            nc.sync.dma_start(out=dg_sb[:], in_=dg_d.ap())
            wn_sb = meta.tile([K_in, F], dt.bfloat16)
            nc.sync.dma_start(out=wn_sb[:], in_=wn_d.ap())
            idn_sb = meta.tile([P, P], dt.bfloat16)
            nc.sync.dma_start(out=idn_sb[:], in_=idn_d.ap())
            iota_t = meta.tile([P, 1, P], dt.bfloat16)
            nc.sync.dma_start(out=iota_t[:, 0, :],
                              in_=iot_d.ap()[0:1, :].partition_broadcast(P))
            web = meta.tile([P, 1, F], dt.float32)
            nc.sync.dma_start(out=web[:],
                              in_=we_d.ap()[0:1, :].partition_broadcast(P))
            bbb = meta.tile([P, 1, F], dt.float32)
            nc.sync.dma_start(out=bbb[:],
                              in_=bb_d.ap()[0:1, :].partition_broadcast(P))

            nc.sync.dma_start(out=idx_sb[:, RCOL:],
                              in_=idx_d.ap()[:, RCOL:])
            acc = meta.tile([P, TILES, K_in], dt.bfloat16)
            efs = meta.tile([P, TILES, 1], dt.float32)
            osb = meta.tile([P, TILES, F], dt.float32)

            # EF[dst] = sum of this dst's edge_feat values (grid is
            # host-placed, zero-padded to LMAX per node).
            nc.vector.tensor_reduce(out=efs[:], in_=efg_sb[:],
                                    axis=mybir.AxisListType.X,
                                    op=mybir.AluOpType.add)

            qc = 0
            col = 0
            for r in range(NR):
                st = pst.tile([P, RBLK, K_in], dt.bfloat16, tag="st")
                for g in range(NGRP):
                    n_g = R * B4 * P
                    g0 = st_blk(g, 0, 0)
                    base = g * CH
                    for s0 in range(0, n_g, SUB):
                        ns = min(SUB, n_g - s0)
                        nc.gpsimd.dma_gather(
                            out_ap=st[:, g0 + s0 // P: g0 + (s0 + ns) // P, :],
                            in_ap=nfr_d.ap()[base:PAD_N, :],
                            idxs_ap=idx_sb[:, col + s0 // 16:
                                           col + (s0 + ns) // 16],
                            num_idxs=ns, num_idxs_reg=ns,
                            elem_size=K_in,
                            queue_num=qc % NQ,
                            single_packet=(ns <= 1024))
                        qc += 1
                    col += n_g // 16
                for tt in range(R):
                    t = r * R + tt
                    oh = poh.tile([P, TPB, P], dt.bfloat16, tag="oh")
                    nc.vector.tensor_tensor(
                        out=oh[:],
                        in0=dstl_sb[:, t, :, :].to_broadcast([P, TPB, P]),
                        in1=iota_t[:].to_broadcast([P, TPB, P]),
                        op=is_equal)
                    ps = pps.tile([P, K_in], dt.float32, tag="ps")
                    for j in range(TPB):
                        g, b = divmod(j, B4)
                        nc.tensor.matmul(
                            ps[:],
                            lhsT=oh[:, j, :],
                            rhs=st[:, st_blk(g, tt, b), :],
                            start=(j == 0), stop=(j == TPB - 1))
                    nc.scalar.copy(out=acc[:, t, :], in_=ps[:])
                    # finalize tile t inline: S_raw^T then @ W_node
                    psT = ppsT.tile([P, P], dt.float32, tag="psT")
                    nc.tensor.matmul(psT[:], lhsT=acc[:, t, :], rhs=idn_sb[:],
                                     start=True, stop=True)
                    hT = pfin.tile([P, P], dt.bfloat16, tag="hT")
                    nc.scalar.copy(out=hT[:], in_=psT[:])
                    pso = ppso.tile([P, F], dt.float32, tag="pso")
                    nc.tensor.matmul(pso[:], lhsT=hT[:], rhs=wn_sb[:],
                                     start=True, stop=True)
                    nc.scalar.copy(out=osb[:, t, :], in_=pso[:])
                # per-round epilogue + output store:
                # out = (S@W + EF*We + deg*(bn+be)) * rdeg
                t0 = r * R
                sl = slice(t0, t0 + R)
                t1 = pfin.tile([P, R, F], dt.float32, tag="t1")
                nc.vector.tensor_tensor(
                    out=t1[:],
                    in0=efs[:, sl, :].to_broadcast([P, R, F]),
                    in1=web[:].to_broadcast([P, R, F]),
                    op=mult)
                nc.vector.tensor_add(out=osb[:, sl, :], in0=osb[:, sl, :],
                                     in1=t1[:])
                nc.vector.tensor_tensor(
                    out=t1[:],
                    in0=dg_sb[:, sl, 0:1].to_broadcast([P, R, F]),
                    in1=bbb[:].to_broadcast([P, R, F]),
                    op=mult)
                nc.vector.tensor_add(out=osb[:, sl, :], in0=osb[:, sl, :],
                                     in1=t1[:])
                nc.vector.tensor_tensor(
                    out=osb[:, sl, :], in0=osb[:, sl, :],
                    in1=dg_sb[:, sl, 1:2].to_broadcast([P, R, F]),
                    op=mult)
                nc.sync.dma_start(
                    out=out_d.ap().rearrange("(p t) f -> p t f",
                                             t=TILES)[:, sl, :],
                    in_=osb[:, sl, :])
    nc.compile()
    return nc


def _schedule(src, dst, edge_feat, n_nodes):
    """Host-side index-space binning by (core, dst-tile, src-quarter)."""
    RN = n_nodes // N_CORES
    TILES = (RN + P - 1) // P
    R = 1
    for d in range(1, TILES + 1):
        if TILES % d == 0 and d <= 7:
            R = d
    NR = TILES // R
    PAD_N = -(-n_nodes // P) * P
    CH = -(-PAD_N // NGRP)
    assert CH <= 32768

    core = dst // RN
    L = dst - core * RN
    t = L // P
    u = (L % P).astype(np.float32)
    g = src // CH
    key = (core * TILES + t) * NGRP + g
    order = np.lexsort((src, key))
    ks, ss, us = key[order], src[order], u[order]
    nbins = N_CORES * TILES * NGRP
    cnt = np.bincount(key, minlength=nbins)
    starts = np.zeros(nbins + 1, dtype=np.int64)
    np.cumsum(cnt, out=starts[1:])
    B4 = max(1, int(np.ceil(cnt.max() / P)))
    TPB = NGRP * B4
    SLOTS_T = TPB * P

    per_core = []
    for c in range(N_CORES):
        idxv = np.zeros(TILES * SLOTS_T, dtype=np.int16)
        dstl = np.full((TILES, TPB, P), -1.0, dtype=np.float32)
        pos = 0
        # gather-stream order: [round][quarter][tile][B4*P slots]
        for r in range(NR):
            for gg in range(NGRP):
                for tt in range(R):
                    bi = (c * TILES + r * R + tt) * NGRP + gg
                    a, b = starts[bi], starts[bi + 1]
                    n = b - a
                    idxv[pos:pos + n] = (ss[a:b] - gg * CH).astype(np.int16)
                    blkv = np.full(B4 * P, -1.0, dtype=np.float32)
                    blkv[:n] = us[a:b]
                    dstl[r * R + tt, gg * B4:(gg + 1) * B4, :] = \
                        blkv.reshape(B4, P)
                    pos += B4 * P
        per_core.append((idxv, dstl.transpose(2, 0, 1).astype(BF16).copy()))
    return per_core, TILES, R, B4, TPB, RN, PAD_N, CH


def _pack_idx(idxv):
    """flat slot-ordered int16 idxs -> wrapped [P, n/16] (16-partition
    wrap, replicated to the 8 16-partition groups)."""
    w = idxv.reshape(-1, 16).T           # [16, n/16]
    return np.tile(w, (8, 1)).astype(np.int16)


def _run(node_feat, edge_feat, W_node, b_node, W_edge, b_edge, src, dst,
         trace=False):
    n_nodes, K_in = node_feat.shape
    F = W_node.shape[1]
    src = np.asarray(src, dtype=np.int64)
    dst = np.asarray(dst, dtype=np.int64)
    E = src.shape[0]

    per_core, TILES, R, B4, TPB, NR, PAD_N, CH = \
        _schedule(src, dst, edge_feat, n_nodes)
    RN = n_nodes // N_CORES

    nfr = np.zeros((PAD_N, K_in), dtype=BF16)
    nfr[:n_nodes] = node_feat.astype(BF16)

    # per-dst ef grid + degree (host index metadata). Device row of
    # global node n is core*(TILES*P) + (n - core*RN) -- note the
    # per-core tile padding (TILES*P >= RN).
    deg = np.bincount(dst, minlength=n_nodes).astype(np.int64)
    LMAX = max(1, int(deg.max()))
    do = np.argsort(dst, kind="stable")
    dstart = np.zeros(n_nodes + 1, dtype=np.int64)
    np.cumsum(deg, out=dstart[1:])
    rank = np.arange(E, dtype=np.int64) - dstart[dst[do]]
    nn = np.arange(n_nodes, dtype=np.int64)
    pos = (nn // RN) * (TILES * P) + nn % RN      # node -> device row
    grid = np.zeros((N_CORES * TILES * P, LMAX), dtype=np.float32)
    grid[pos[dst[do]], rank] = edge_feat[do, 0]
    degp = np.zeros(N_CORES * TILES * P, dtype=np.float32)
    degp[pos] = deg

    nc = build_bass(K_in, F, TILES, R, B4, LMAX, PAD_N, CH)

    base_in = {
        "nfr": nfr,
        "wn": W_node.astype(BF16),
        "idn": np.eye(P, dtype=np.float32).astype(BF16),
        "iot": np.arange(P, dtype=np.float32).reshape(1, P).astype(BF16),
        "we": W_edge.astype(np.float32).reshape(1, F),
        "bb": (b_node + b_edge).astype(np.float32).reshape(1, F),
    }
    in_maps = []
    for c in range(N_CORES):
        idxv, dstp = per_core[c]
        m = dict(base_in)
        m["idx"] = _pack_idx(idxv)
        m["dstl"] = dstp
        gr = grid[c * TILES * P:(c + 1) * TILES * P]
        m["efg"] = gr.reshape(TILES, P, LMAX).transpose(1, 0, 2) \
                     .astype(BF16).copy()
        d = degp[c * TILES * P:(c + 1) * TILES * P].reshape(TILES, P)
        dgm = np.stack([d.T, 1.0 / np.maximum(d.T, 1.0)], axis=-1)
        m["dg"] = np.ascontiguousarray(dgm.astype(np.float32))
        in_maps.append(m)

    res = run_bass_kernel_spmd(nc, in_maps, core_ids=list(range(N_CORES)),
                               trace=trace)
    loc = np.arange(RN, dtype=np.int64)
    rows = (loc % P) * TILES + loc // P
    out = np.empty((n_nodes, F), dtype=np.float32)
    for c in range(N_CORES):
        out[c * RN:(c + 1) * RN] = res.results[c]["out"][rows]
    return out, res


def kernel(node_feat, edge_feat, W_node, b_node, W_edge, b_edge, src, dst):
    out, _ = _run(node_feat, edge_feat, W_node, b_node, W_edge, b_edge,
                  src, dst)
    return out


# revision 9
# speedup vs baseline: 4.3287x; 1.3003x over previous
"""EdgeGraphConv on 8 Trainium2 NeuronCores (v4: raw-feature gather,
capped bins + pooled overflow).

Distribution: dst-range sharding. Core c owns destination nodes
[c*N/8, (c+1)*N/8). No collectives; output is a concatenation.

Linearity trick: mean_e(h0[src]+bn + ef*We+be) over dst =
  (S_raw @ W + EF*We + deg*(bn+be)) / max(deg,1)
where S_raw[dst] = sum nf[src_e]  (raw 128-dim features!),
      EF[dst]    = sum ef_e,  deg = in-degree (host metadata).
There is NO per-node pre-matmul: the kernel gathers raw node-feature
rows (256 B, fully used) and applies W_node per dst tile AFTER
aggregation (2 tiny matmuls per tile).

Gather: SWDGE dma_gather of 256B rows, round-robin over 4 SWDGE
queues; ~2.6 ns/descriptor, so descriptor COUNT is the wall. Binning
caps each (tile, src-quarter) bin at CAPB*128 slots; the overflow
edges (~2% of total) are pooled per (round, quarter) into BO blocks,
cutting slot padding from ~20% to ~7%. Overflow slots carry a
round-local dst label tt*128+u (fp16 keeps ints <= 2048 exact) and
match against R shifted iota rows in one DVE is_equal.

Single NEFF serves all 8 cores (per-core differences are pure data).
"""

import sys

for _p in ("/opt/trn_rl_repo", "/opt/pypackages"):
    if _p not in sys.path:
        sys.path.append(_p)

import ml_dtypes
import numpy as np

import concourse.bass as bass
import concourse.mybir as mybir
import concourse.tile as tile
from concourse import bacc, library_config
from concourse.bass_utils import run_bass_kernel_spmd

FP16 = np.float16
N_CORES = 8
P = 128
NGRP = 4           # src-value quarters (windows of CH rows)
CAPB = 4           # main blocks per (tile, quarter)
SUB = 2048         # idxs per dma_gather call (128 descs/engine)
NQ = 4             # SWDGE queues


def build_bass(K_in, F, TILES, R, BO, LMAX, PAD_N, CH):
    NR = TILES // R
    TPB = NGRP * CAPB                        # main blocks per tile
    GB = R * CAPB + BO                       # st blocks per quarter-round
    RBLK = NGRP * GB                         # st blocks per round
    SLOTS_R = RBLK * P                       # slots per round
    TOT_SLOTS = NR * SLOTS_R

    nc = bacc.Bacc("TRN2", target_bir_lowering=False, debug=False,
                   num_devices=N_CORES, num_swdge_queues=NQ)
    dt = mybir.dt

    nfr_d = nc.dram_tensor("nfr", [PAD_N, K_in], dt.float16, kind="ExternalInput")
    wn_d = nc.dram_tensor("wn", [K_in, F], dt.float16, kind="ExternalInput")
    idn_d = nc.dram_tensor("idn", [P, P], dt.float16, kind="ExternalInput")
    iot_d = nc.dram_tensor("iot", [R, P], dt.float16, kind="ExternalInput")
    we_d = nc.dram_tensor("we", [1, F], dt.float32, kind="ExternalInput")
    bb_d = nc.dram_tensor("bb", [1, F], dt.float32, kind="ExternalInput")
    idx_d = nc.dram_tensor("idx", [P, TOT_SLOTS // 16], dt.int16,
                           kind="ExternalInput")
    dstl_d = nc.dram_tensor("dstl", [P, TILES, TPB], dt.float16,
                            kind="ExternalInput")
    dsto_d = nc.dram_tensor("dsto", [P, NR, NGRP * BO], dt.float16,
                            kind="ExternalInput")
    efg_d = nc.dram_tensor("efg", [P, TILES, LMAX], dt.float16,
                           kind="ExternalInput")
    dg_d = nc.dram_tensor("dg", [P, TILES, 2], dt.float32, kind="ExternalInput")
    out_d = nc.dram_tensor("out", [TILES * P, F], dt.float32,
                           kind="ExternalOutput")

    mult = mybir.AluOpType.mult
    is_equal = mybir.AluOpType.is_equal

    def st_main(g, tt, b):
        return g * GB + tt * CAPB + b

    def st_ovf(g, b):
        return g * GB + R * CAPB + b

    with tile.TileContext(nc) as tc:
        with tc.tile_pool(name="meta", bufs=1) as meta, \
             tc.tile_pool(name="st", bufs=2) as pst, \
             tc.tile_pool(name="oh", bufs=4) as poh, \
             tc.tile_pool(name="oho", bufs=2) as poho, \
             tc.tile_pool(name="fin", bufs=2) as pfin, \
             tc.tile_pool(name="ps", bufs=3, space="PSUM") as pps, \
             tc.tile_pool(name="psT", bufs=2, space="PSUM") as ppsT, \
             tc.tile_pool(name="pso", bufs=2, space="PSUM") as ppso:
            nc.gpsimd.load_library(library_config.mlp)
            RCOL = SLOTS_R // 16          # idx columns per round
            idx_sb = meta.tile([P, TOT_SLOTS // 16], dt.int16)
            nc.sync.dma_start(out=idx_sb[:, 0:RCOL], in_=idx_d.ap()[:, 0:RCOL])
            dstl_sb = meta.tile([P, TILES, TPB, 1], dt.float16)
            nc.sync.dma_start(out=dstl_sb[:, :, :, 0], in_=dstl_d.ap())
            dsto_sb = meta.tile([P, NR, NGRP * BO, 1, 1], dt.float16)
            nc.sync.dma_start(out=dsto_sb[:, :, :, 0, 0], in_=dsto_d.ap())
            efg_sb = meta.tile([P, TILES, LMAX], dt.float16)
            nc.sync.dma_start(out=efg_sb[:], in_=efg_d.ap())
            dg_sb = meta.tile([P, TILES, 2], dt.float32)
            nc.sync.dma_start(out=dg_sb[:], in_=dg_d.ap())
            wn_sb = meta.tile([K_in, F], dt.float16)
            nc.sync.dma_start(out=wn_sb[:], in_=wn_d.ap())
            idn_sb = meta.tile([P, P], dt.float16)
            nc.sync.dma_start(out=idn_sb[:], in_=idn_d.ap())
            # iota rows: iota_t[p, 0, tt, i] = tt*128 + i
            iota_t = meta.tile([P, 1, R, P], dt.float16)
            for tt in range(R):
                nc.sync.dma_start(
                    out=iota_t[:, 0, tt, :],
                    in_=iot_d.ap()[tt:tt + 1, :].partition_broadcast(P))
            web = meta.tile([P, 1, F], dt.float32)
            nc.sync.dma_start(out=web[:],
                              in_=we_d.ap()[0:1, :].partition_broadcast(P))
            bbb = meta.tile([P, 1, F], dt.float32)
            nc.sync.dma_start(out=bbb[:],
                              in_=bb_d.ap()[0:1, :].partition_broadcast(P))
            nc.sync.dma_start(out=idx_sb[:, RCOL:],
                              in_=idx_d.ap()[:, RCOL:])
            acc = meta.tile([P, TILES, K_in], dt.float16)
            efs = meta.tile([P, TILES, 1], dt.float32)
            osb = meta.tile([P, TILES, F], dt.float32)

            # EF[dst] = sum of this dst's edge_feat values
            nc.vector.tensor_reduce(out=efs[:], in_=efg_sb[:],
                                    axis=mybir.AxisListType.X,
                                    op=mybir.AluOpType.add)

            qc = 0
            for r in range(NR):
                st = pst.tile([P, RBLK, K_in], dt.float16, tag="st")
                col = r * RCOL
                for g in range(NGRP):
                    n_g = GB * P
                    g0 = g * GB
                    base = g * CH
                    for s0 in range(0, n_g, SUB):
                        ns = min(SUB, n_g - s0)
                        nc.gpsimd.dma_gather(
                            out_ap=st[:, g0 + s0 // P: g0 + (s0 + ns) // P, :],
                            in_ap=nfr_d.ap()[base:PAD_N, :],
                            idxs_ap=idx_sb[:, col + s0 // 16:
                                           col + (s0 + ns) // 16],
                            num_idxs=ns, num_idxs_reg=ns,
                            elem_size=K_in,
                            queue_num=qc % NQ,
                            single_packet=(ns <= 1024))
                        qc += 1
                    col += n_g // 16
                # overflow one-hots for this round, one DVE op:
                # oho[p, g, b, tt, i] = (dsto[p, r, g, b] == tt*128+i)
                oho = poho.tile([P, NGRP * BO, R, P], dt.float16, tag="oho")
                nc.vector.tensor_tensor(
                    out=oho[:],
                    in0=dsto_sb[:, r, :, :, :].to_broadcast(
                        [P, NGRP * BO, R, P]),
                    in1=iota_t[:].to_broadcast([P, NGRP * BO, R, P]),
                    op=is_equal)
                for tt in range(R):
                    t = r * R + tt
                    oh = poh.tile([P, TPB, P], dt.float16, tag="oh")
                    nc.vector.tensor_tensor(
                        out=oh[:],
                        in0=dstl_sb[:, t, :, :].to_broadcast([P, TPB, P]),
                        in1=iota_t[:, :, 0, :].to_broadcast([P, TPB, P]),
                        op=is_equal)
                    ps = pps.tile([P, K_in], dt.float32, tag="ps")
                    for j in range(TPB):
                        g, b = divmod(j, CAPB)
                        nc.tensor.matmul(
                            ps[:],
                            lhsT=oh[:, j, :],
                            rhs=st[:, st_main(g, tt, b), :],
                            start=(j == 0), stop=False)
                    for g in range(NGRP):
                        for b in range(BO):
                            last = (g == NGRP - 1 and b == BO - 1)
                            nc.tensor.matmul(
                                ps[:],
                                lhsT=oho[:, g * BO + b, tt, :],
                                rhs=st[:, st_ovf(g, b), :],
                                start=False, stop=last)
                    nc.scalar.copy(out=acc[:, t, :], in_=ps[:])
                    # finalize tile t inline: S_raw^T then @ W_node
                    psT = ppsT.tile([P, P], dt.float32, tag="psT")
                    nc.tensor.matmul(psT[:], lhsT=acc[:, t, :], rhs=idn_sb[:],
                                     start=True, stop=True)
                    hT = pfin.tile([P, P], dt.float16, tag="hT")
                    nc.scalar.copy(out=hT[:], in_=psT[:])
                    pso = ppso.tile([P, F], dt.float32, tag="pso")
                    nc.tensor.matmul(pso[:], lhsT=hT[:], rhs=wn_sb[:],
                                     start=True, stop=True)
                    nc.scalar.copy(out=osb[:, t, :], in_=pso[:])
                # per-round epilogue + output store:
                # out = (S@W + EF*We + deg*(bn+be)) * rdeg
                t0 = r * R
                sl = slice(t0, t0 + R)
                t1 = pfin.tile([P, R, F], dt.float32, tag="t1")
                nc.vector.tensor_tensor(
                    out=t1[:],
                    in0=efs[:, sl, :].to_broadcast([P, R, F]),
                    in1=web[:].to_broadcast([P, R, F]),
                    op=mult)
                nc.vector.tensor_add(out=osb[:, sl, :], in0=osb[:, sl, :],
                                     in1=t1[:])
                nc.vector.tensor_tensor(
                    out=t1[:],
                    in0=dg_sb[:, sl, 0:1].to_broadcast([P, R, F]),
                    in1=bbb[:].to_broadcast([P, R, F]),
                    op=mult)
                nc.vector.tensor_add(out=osb[:, sl, :], in0=osb[:, sl, :],
                                     in1=t1[:])
                nc.vector.tensor_tensor(
                    out=osb[:, sl, :], in0=osb[:, sl, :],
                    in1=dg_sb[:, sl, 1:2].to_broadcast([P, R, F]),
                    op=mult)
                nc.sync.dma_start(
                    out=out_d.ap().rearrange("(p t) f -> p t f",
                                             t=TILES)[:, sl, :],
                    in_=osb[:, sl, :])
    nc.compile()
    return nc


def _schedule(src, dst, edge_feat, n_nodes):
    """Host-side index-space binning by (core, dst-tile, src-quarter)
    with per-bin cap CAPB*128 and pooled per-(round, quarter) overflow."""
    RN = n_nodes // N_CORES
    TILES = (RN + P - 1) // P
    R = 1
    for d in range(1, TILES + 1):
        if TILES % d == 0 and d <= 7:
            R = d
    NR = TILES // R
    PAD_N = -(-n_nodes // P) * P
    CH = -(-PAD_N // NGRP)
    assert CH <= 32768

    core = dst // RN
    L = dst - core * RN
    t = L // P
    u = (L % P).astype(np.float32)
    g = src // CH
    key = (core * TILES + t) * NGRP + g
    order = np.lexsort((src, key))
    ss, us = src[order], u[order]
    nbins = N_CORES * TILES * NGRP
    cnt = np.bincount(key, minlength=nbins)
    starts = np.zeros(nbins + 1, dtype=np.int64)
    np.cumsum(cnt, out=starts[1:])
    CAP = CAPB * P
    # pooled overflow size per (core, round, quarter)
    ovf = np.maximum(cnt.reshape(N_CORES, TILES, NGRP) - CAP, 0)
    po = ovf.reshape(N_CORES, NR, R, NGRP).sum(axis=2)
    BO = max(1, int(np.ceil(po.max() / P)))

    GB = R * CAPB + BO
    SLOTS_R = NGRP * GB * P
    per_core = []
    for c in range(N_CORES):
        idxv = np.zeros(NR * SLOTS_R, dtype=np.int16)
        dstl = np.full((TILES, NGRP * CAPB, P), -1.0, dtype=np.float32)
        dsto = np.full((NR, NGRP, BO * P), -1.0, dtype=np.float32)
        for r in range(NR):
            for gg in range(NGRP):
                # main slots: [tile][CAP], then overflow pool [BO*P]
                p0 = r * SLOTS_R + gg * GB * P
                op_ = p0 + R * CAP
                no = 0
                for tt in range(R):
                    bi = (c * TILES + r * R + tt) * NGRP + gg
                    a, b = starts[bi], starts[bi + 1]
                    n = b - a
                    nm = min(n, CAP)
                    idxv[p0:p0 + nm] = (ss[a:a + nm] - gg * CH).astype(np.int16)
                    blkv = np.full(CAP, -1.0, dtype=np.float32)
                    blkv[:nm] = us[a:a + nm]
                    dstl[r * R + tt, gg * CAPB:(gg + 1) * CAPB, :] = \
                        blkv.reshape(CAPB, P)
                    if n > CAP:
                        k = n - CAP
                        idxv[op_ + no:op_ + no + k] = \
                            (ss[a + CAP:b] - gg * CH).astype(np.int16)
                        dsto[r, gg, no:no + k] = tt * P + us[a + CAP:b]
                        no += k
                    p0 += CAP
        per_core.append((
            idxv,
            dstl.transpose(2, 0, 1).astype(FP16).copy(),
            dsto.reshape(NR, NGRP * BO, P).transpose(2, 0, 1)
                .astype(FP16).copy(),
        ))
    return per_core, TILES, R, BO, NR, PAD_N, CH


def _pack_idx(idxv):
    """flat slot-ordered int16 idxs -> wrapped [P, n/16] (16-partition
    wrap, replicated to the 8 16-partition groups)."""
    w = idxv.reshape(-1, 16).T           # [16, n/16]
    return np.tile(w, (8, 1)).astype(np.int16)


def _run(node_feat, edge_feat, W_node, b_node, W_edge, b_edge, src, dst,
         trace=False):
    n_nodes, K_in = node_feat.shape
    F = W_node.shape[1]
    src = np.asarray(src, dtype=np.int64)
    dst = np.asarray(dst, dtype=np.int64)
    E = src.shape[0]

    per_core, TILES, R, BO, NR, PAD_N, CH = \
        _schedule(src, dst, edge_feat, n_nodes)
    RN = n_nodes // N_CORES

    nfr = np.zeros((PAD_N, K_in), dtype=FP16)
    nfr[:n_nodes] = node_feat.astype(FP16)

    # per-dst ef grid + degree (host index metadata). Device row of
    # global node n is core*(TILES*P) + (n - core*RN).
    deg = np.bincount(dst, minlength=n_nodes).astype(np.int64)
    LMAX = max(1, int(deg.max()))
    do = np.argsort(dst, kind="stable")
    dstart = np.zeros(n_nodes + 1, dtype=np.int64)
    np.cumsum(deg, out=dstart[1:])
    rank = np.arange(E, dtype=np.int64) - dstart[dst[do]]
    nn = np.arange(n_nodes, dtype=np.int64)
    pos = (nn // RN) * (TILES * P) + nn % RN      # node -> device row
    grid = np.zeros((N_CORES * TILES * P, LMAX), dtype=np.float32)
    grid[pos[dst[do]], rank] = edge_feat[do, 0]
    degp = np.zeros(N_CORES * TILES * P, dtype=np.float32)
    degp[pos] = deg

    nc = build_bass(K_in, F, TILES, R, BO, LMAX, PAD_N, CH)

    iot = np.arange(P, dtype=np.float32).reshape(1, P) + \
        np.arange(R, dtype=np.float32).reshape(R, 1) * P
    base_in = {
        "nfr": nfr,
        "wn": W_node.astype(FP16),
        "idn": np.eye(P, dtype=np.float32).astype(FP16),
        "iot": iot.astype(FP16),
        "we": W_edge.astype(np.float32).reshape(1, F),
        "bb": (b_node + b_edge).astype(np.float32).reshape(1, F),
    }
    in_maps = []
    for c in range(N_CORES):
        idxv, dstp, dsto = per_core[c]
        m = dict(base_in)
        m["idx"] = _pack_idx(idxv)
        m["dstl"] = dstp
        m["dsto"] = dsto
        gr = grid[c * TILES * P:(c + 1) * TILES * P]
        m["efg"] = gr.reshape(TILES, P, LMAX).transpose(1, 0, 2) \
                     .astype(FP16).copy()
        d = degp[c * TILES * P:(c + 1) * TILES * P].reshape(TILES, P)
        dgm = np.stack([d.T, 1.0 / np.maximum(d.T, 1.0)], axis=-1)
        m["dg"] = np.ascontiguousarray(dgm.astype(np.float32))
        in_maps.append(m)

    res = run_bass_kernel_spmd(nc, in_maps, core_ids=list(range(N_CORES)),
                               trace=trace)
    loc = np.arange(RN, dtype=np.int64)
    rows = (loc % P) * TILES + loc // P
    out = np.empty((n_nodes, F), dtype=np.float32)
    for c in range(N_CORES):
        out[c * RN:(c + 1) * RN] = res.results[c]["out"][rows]
    return out, res


def kernel(node_feat, edge_feat, W_node, b_node, W_edge, b_edge, src, dst):
    out, _ = _run(node_feat, edge_feat, W_node, b_node, W_edge, b_edge,
                  src, dst)
    return out
